# revision 4
# baseline (speedup 1.0000x reference)
"""TRN2 Bass kernel for nn_PosOnlyModel: 2-layer LSTM encoder (15 steps) +
autoregressive 2-layer LSTM decoder (25 steps) with Linear head and
unit-sphere residual position updates.

Pure data parallel across 8 NeuronCores (batch 8192 -> 1024/core, weights
replicated). Per-core structure:
- Recurrent matmuls (Whh0, Wih1, Whh1) run in fp8e4 DoubleRow mode: K=256 in
  ONE PE pass (half the streaming cycles and half the matmul dispatch of an
  fp16 2-pass kernel). Weights are packed x16 and h stored as 64*h in fp8;
  the gate activation applies scale=1/1024 for free.
- Layer-0 bias folds into the K=4 fp16 x-pass via a ones row in the x tile.
- g-gate rows carry an extra x2 so every gate uses Sigmoid; tanh(g) is
  recovered as 2*sigmoid(2g)-1 by one DVE tensor_scalar (4x mode). The
  g sigmoid is kept fp32 to avoid cancellation noise near 0.5.
- h = (64*sigmoid(o)) * tanh(c): the 64*sigmoid(o) tensor_scalar runs off
  the critical path, so the step tail is ACT-tanh + one TT into fp8.
- Per-gate PSUM tiles [128,1024] (2 banks, bufs=4) pipeline PE->ACT;
  element-wise cell updates run in 512-wide chunks so consecutive steps
  overlap across engines. The whole c-update (m1, tg, m2, add) stays on DVE
  in fp16 - keeping it on one engine removes cross-engine semaphore hops
  from the recurrence-critical path (moving m1 to Pool measured 6% slower).
- Decoder position chain: batch-major [128, 8, 3] DVE ops with bit-trick
  Newton rsqrt (ACT Sqrt would force a ~1.3us activation-table swap per
  step), PE transposes back to [3, B], and the PSUM->SBUF evacuation split
  across ACT and DVE.
"""
import sys
import numpy as np
import ml_dtypes

sys.path.insert(0, '/opt/trn_rl_repo')

import concourse.bass as bass
import concourse.bacc as bacc
import concourse.mybir as mybir
from concourse.tile import TileContext

dt = mybir.dt
F32 = dt.float32
F16 = dt.float16
E4 = dt.float8e4
U32 = dt.uint32
AF = mybir.ActivationFunctionType
ALU = mybir.AluOpType
AX = mybir.AxisListType
DR = mybir.MatmulPerfMode.DoubleRow

H = 256
T_ENC = 15
T_DEC = 25
N_CORES = 8
B_FULL = 8192
B = B_FULL // N_CORES
NB = B // 128
NK = 2
NC = 2          # batch chunks for element-wise pipelining
CW = B // NC    # chunk width
WS = 16.0       # weight scale into fp8 range
HS = 64.0       # h scale into fp8 range (h stored as 64*h)
FS = 16.0       # fc weight scale
RSQRT_MAGIC = 0x5f3759df
E4NP = ml_dtypes.float8_e4m3


def build_kernel(repeats=1):
    nc = bacc.Bacc("TRN2", target_bir_lowering=False)

    enc_x = nc.declare_dram_parameter("enc_x", [4, T_ENC * B], F16, isOutput=False)
    pos0 = nc.declare_dram_parameter("pos0", [4, B], F16, isOutput=False)
    pos0bm = nc.declare_dram_parameter("pos0bm", [128, NB * 3], F32, isOutput=False)
    w8_d = {}
    for name in ("whh_e0", "wih_e1", "whh_e1", "whh_d0", "wih_d1", "whh_d1"):
        w8_d[name] = nc.declare_dram_parameter(name, [128, NK * 8 * 128], E4, isOutput=False)
    wx_d = {c: nc.declare_dram_parameter(f"wx_{c}", [4, 8 * 128], F16, isOutput=False)
            for c in ("e0", "d0")}
    bias1_d = nc.declare_dram_parameter("bias1", [128, 16], F32, isOutput=False)
    fcw_d = nc.declare_dram_parameter("fcw", [128, NK * 4], E4, isOutput=False)
    fcbbm_d = nc.declare_dram_parameter("fcbbm", [128, NB * 3], F32, isOutput=False)
    magic_d = nc.declare_dram_parameter("magic", [128, NB], U32, isOutput=False)
    ident_d = nc.declare_dram_parameter("ident", [128, 128], F32, isOutput=False)
    ys_d = nc.declare_dram_parameter("ys", [128, T_DEC * NB * 3], F32, isOutput=True)

    with TileContext(nc) as tc:
        with tc.tile_pool(name="wpool", bufs=1) as wp, \
             tc.tile_pool(name="state", bufs=1) as sp, \
             tc.tile_pool(name="acts", bufs=12) as ap, \
             tc.tile_pool(name="tanhs", bufs=4) as t3p, \
             tc.tile_pool(name="tmps", bufs=8) as tp, \
             tc.tile_pool(name="xp", bufs=3) as xp, \
             tc.tile_pool(name="dec", bufs=3) as dp, \
             tc.tile_pool(name="psum", bufs=4, space="PSUM") as pp:

            w8 = {}
            for name in w8_d:
                w8[name] = wp.tile([128, NK, 8 * 128], E4, name=name)
                nc.sync.dma_start(out=w8[name][:],
                                  in_=w8_d[name][:].rearrange("p (k m) -> p k m", k=NK))
            wx = {}
            for c in ("e0", "d0"):
                wx[c] = wp.tile([4, 8 * 128], F16, name=f"wx_{c}")
                nc.gpsimd.dma_start(out=wx[c][:], in_=wx_d[c][:])
            bias1_t = wp.tile([128, 16], F32, name="bias1")
            nc.gpsimd.dma_start(out=bias1_t[:], in_=bias1_d[:])
            fcw_t = wp.tile([128, NK, 4], E4, name="fcw")
            nc.gpsimd.dma_start(out=fcw_t[:], in_=fcw_d[:].rearrange("p (k m) -> p k m", k=NK))
            fcbbm_t = wp.tile([128, NB * 3], F32, name="fcbbm")
            nc.gpsimd.dma_start(out=fcbbm_t[:], in_=fcbbm_d[:])
            magic_t = wp.tile([128, NB], U32, name="magic")
            nc.gpsimd.dma_start(out=magic_t[:], in_=magic_d[:])
            ident_t = wp.tile([128, 128], F32, name="ident")
            nc.gpsimd.dma_start(out=ident_t[:], in_=ident_d[:])
            pos0bm_t = wp.tile([128, NB * 3], F32, name="pos0bm")
            nc.gpsimd.dma_start(out=pos0bm_t[:], in_=pos0bm[:])
            posx = [wp.tile([4, B], F16, name=f"posx{i}") for i in range(2)]
            for t in posx:
                nc.gpsimd.dma_start(out=t[3:4, :], in_=pos0[3:4, :])

            h_t = [[sp.tile([128, NK, B], E4, name=f"h{l}_{par}") for par in range(2)]
                   for l in range(2)]
            c_t = [sp.tile([128, NK, B], F16, name=f"c{l}") for l in range(2)]

            CELLS = {0: ("e0", "e1"), 1: ("d0", "d1")}

            def emit_cell(cell, layer, s, x_ap, dr_rd, h_wr, c_rw, bias_base):
                """dr_rd: list of (w8_tile, h_rhs_tile) DoubleRow passes.
                Layer 0 adds the fp16 x-pass (bias folded via ones row)."""
                first = s == 0
                ga = {}  # gate -> act tile AP, key (hb, mg)
                so64 = {}
                for hb in range(2):
                    # f first (Pool m1 overlaps), g second (tg->m2->add chain
                    # starts early), o last (h-mul is its only consumer)
                    for mg in (1, 2, 0, 3):
                        m = hb * 4 + mg
                        msl = slice(m * 128, (m + 1) * 128)
                        ps = pp.tile([128, B], F32, name="ps", tag="ps")
                        n_pass = len(dr_rd) + (1 if layer == 0 else 0)
                        pj = 0
                        for (w, rhs) in dr_rd:
                            for ci in range(NC):
                                cs = slice(ci * CW, (ci + 1) * CW)
                                nc.tensor.matmul(ps[:, cs], w[:, :, msl], rhs[:, :, cs],
                                                 start=(pj == 0), stop=(pj == n_pass - 1),
                                                 perf_mode=DR)
                            pj += 1
                        if layer == 0:
                            for ci in range(NC):
                                cs = slice(ci * CW, (ci + 1) * CW)
                                nc.tensor.matmul(ps[:, cs], wx[cell][0:4, msl],
                                                 x_ap[0:4, cs],
                                                 start=(pj == 0), stop=True)
                        # g-gate sigmoid kept fp32: tg = 2*sig-1 cancels near
                        # 0.5, so fp16 storage would add ~5e-4 abs noise/step
                        a = ap.tile([128, B], F32 if mg == 2 else F16,
                                    name="act", tag="actg" if mg == 2 else "act")
                        if layer == 0:
                            nc.scalar.activation(a[:], ps[:], AF.Sigmoid,
                                                 scale=1.0 / (WS * HS))
                        else:
                            col = bias_base + m
                            nc.scalar.activation(a[:], ps[:], AF.Sigmoid,
                                                 bias=bias1_t[:, col:col + 1],
                                                 scale=1.0 / (WS * HS))
                        ga[(hb, mg)] = a
                    # HS*sig(o) off the critical path (exact: HS is 2^6)
                    for ci in range(NC):
                        cs = slice(ci * CW, (ci + 1) * CW)
                        so = tp.tile([128, CW], F16, name="so64", tag="so64")
                        nc.vector.tensor_scalar(out=so[:], in0=ga[(hb, 3)][:, cs],
                                                scalar1=HS, scalar2=None, op0=ALU.mult)
                        so64[(hb, ci)] = so
                    # chunked cell update for this half
                    for ci in range(NC):
                        cs = slice(ci * CW, (ci + 1) * CW)
                        cd = c_rw[:, hb, cs]
                        tg = tp.tile([128, CW], F16, name="tg", tag="tg")
                        nc.vector.tensor_scalar(out=tg[:], in0=ga[(hb, 2)][:, cs],
                                                scalar1=2.0, scalar2=-1.0,
                                                op0=ALU.mult, op1=ALU.add)
                        if first:
                            nc.vector.tensor_mul(cd, ga[(hb, 0)][:, cs], tg[:])
                        else:
                            # m1 on DVE (not Pool): f activates first so m1 is
                            # ready early, and keeping the whole c-update on
                            # one engine removes the Pool->DVE semaphore hop
                            # from the recurrence-critical path
                            m1 = tp.tile([128, CW], F16, name="m1", tag="m1")
                            nc.vector.tensor_mul(m1[:], ga[(hb, 1)][:, cs], cd)
                            m2 = tp.tile([128, CW], F16, name="m2", tag="m2")
                            nc.vector.tensor_mul(m2[:], ga[(hb, 0)][:, cs], tg[:])
                            nc.vector.tensor_add(cd, m1[:], m2[:])
                # h = (HS*sig(o)) * tanh(c) -> fp8 stores 64h; tanh feeds the
                # h-mul directly so the step tail is just ACT-tanh + one TT
                for ci in range(NC):
                    cs = slice(ci * CW, (ci + 1) * CW)
                    th = t3p.tile([128, NK, CW], F16, name="th", tag="th")
                    nc.scalar.activation(th[:], c_rw[:, :, cs], AF.Tanh)
                    for hb in range(2):
                        nc.vector.tensor_mul(h_wr[:, hb, cs], so64[(hb, ci)][:],
                                             th[:, hb, :])

            for rep in range(repeats):
                pos_bm = None
                for s in range(T_ENC + T_DEC):
                    is_dec = s >= T_ENC
                    d = s - T_ENC
                    p, q = s % 2, 1 - s % 2
                    c0n, c1n = CELLS[1 if is_dec else 0]
                    first = s == 0
                    if is_dec:
                        if d == 0:
                            nc.sync.dma_start(out=posx[0][:], in_=pos0[:])
                            pos_bm = dp.tile([128, NB * 3], F32, name="pbm", tag="pbm")
                            nc.vector.tensor_add(pos_bm[:], pos0bm_t[:], fcbbm_t[:])
                        x_ap = posx[d % 2][:]
                    else:
                        xt = xp.tile([4, B], F16, name="xst", tag="xst")
                        nc.sync.dma_start(out=xt[:], in_=enc_x[:, s * B:(s + 1) * B])
                        x_ap = xt[:]

                    dr0 = [] if first else [(w8["whh_" + c0n], h_t[0][q])]
                    emit_cell(c0n, 0, s, x_ap, dr0, h_t[0][p], c_t[0], 0)
                    # whh pass first: h1(s-1) is available long before h0(s),
                    # so the PE can fill the psum group early
                    dr1 = []
                    if not first:
                        dr1.append((w8["whh_" + c1n], h_t[1][q]))
                    dr1.append((w8["wih_" + c1n], h_t[0][p]))
                    emit_cell(c1n, 1, s, None, dr1, h_t[1][p], c_t[1],
                              0 if not is_dec else 8)

                    if is_dec:
                        psd = pp.tile([128, NB, 4], F32, name="psd", tag="ps")
                        for m in range(NB):
                            for k in range(NK):
                                nc.tensor.matmul(psd[:, m, :],
                                                 h_t[1][p][:, k, m * 128:(m + 1) * 128],
                                                 fcw_t[:, k, :],
                                                 start=(k == 0), stop=(k == NK - 1))
                        nd = dp.tile([128, NB * 3], F32, name="nd", tag="nd")
                        nc.vector.tensor_scalar(
                            out=nd[:].rearrange("p (m c) -> p m c", c=3),
                            in0=psd[:, :, 0:3], scalar1=1.0 / (HS * FS), scalar2=None,
                            op0=ALU.mult)
                        npos = dp.tile([128, NB * 3], F32, name="npos", tag="npos")
                        nc.vector.tensor_add(npos[:], nd[:], pos_bm[:])
                        sq = dp.tile([128, NB * 3], F32, name="sq", tag="sq")
                        nc.vector.tensor_mul(sq[:], npos[:], npos[:])
                        ss = dp.tile([128, NB], F32, name="ss", tag="ss")
                        nc.vector.tensor_reduce(ss[:], sq[:].rearrange("p (m c) -> p m c", c=3),
                                                axis=AX.X, op=ALU.add)
                        # rsqrt via bit trick + 2 Newton iters. (ACT Sqrt is
                        # accurate but lives in another activation-table set:
                        # using it costs 2 x ~1.3us table loads per step.)
                        u1 = dp.tile([128, NB], U32, name="u1", tag="u1")
                        nc.vector.tensor_scalar(out=u1[:], in0=ss[:].bitcast(U32),
                                                scalar1=1, scalar2=None,
                                                op0=ALU.logical_shift_right)
                        y = dp.tile([128, NB], F32, name="y", tag="y")
                        nc.vector.tensor_tensor(out=y[:].bitcast(U32), in0=magic_t[:],
                                                in1=u1[:], op=ALU.subtract)
                        for _ in range(2):
                            t = dp.tile([128, NB], F32, name="nrt", tag="nrt")
                            nc.vector.tensor_mul(t[:], y[:], y[:])
                            nc.vector.tensor_mul(t[:], t[:], ss[:])
                            nc.vector.tensor_scalar(out=t[:], in0=t[:], scalar1=-0.5,
                                                    scalar2=1.5, op0=ALU.mult, op1=ALU.add)
                            nc.vector.tensor_mul(y[:], y[:], t[:])
                        posn = dp.tile([128, NB * 3], F32, name="posn", tag="posn")
                        yb = y[:].unsqueeze(2).broadcast_to([128, NB, 3])
                        nc.vector.tensor_tensor(out=posn[:].rearrange("p (m c) -> p m c", c=3),
                                                in0=npos[:].rearrange("p (m c) -> p m c", c=3),
                                                in1=yb, op=ALU.mult)
                        nc.sync.dma_start(out=ys_d[:, d * NB * 3:(d + 1) * NB * 3], in_=posn[:])
                        if d < T_DEC - 1:
                            pbm2 = dp.tile([128, NB * 3], F32, name="pbm", tag="pbm")
                            nc.vector.tensor_add(pbm2[:], posn[:], fcbbm_t[:])
                            pos_bm = pbm2
                            ps_tr = pp.tile([3, B], F32, name="ps_tr", tag="ps")
                            for m in range(NB):
                                nc.tensor.transpose(ps_tr[:, m * 128:(m + 1) * 128],
                                                    posn[:, m * 3:(m + 1) * 3], ident_t[:])
                            # split the PSUM->SBUF evacuation across ACT+DVE
                            nxt = posx[(d + 1) % 2]
                            nc.scalar.copy(out=nxt[0:3, 0:CW], in_=ps_tr[:, 0:CW])
                            nc.vector.tensor_copy(out=nxt[0:3, CW:B], in_=ps_tr[:, CW:B])

    nc.finalize()
    return nc


def pack_inputs(inputs):
    perm = np.concatenate([np.arange(g * 256 + hb * 128, g * 256 + hb * 128 + 128)
                           for hb in range(2) for g in range(4)])
    # x2 on the g gate rows so tanh(g) = 2*sigmoid(2g)-1 (all gates Sigmoid)
    rowscale = np.ones(4 * H, np.float32)
    for m in range(8):
        if m % 4 == 2:
            rowscale[m * 128:(m + 1) * 128] = 2.0

    def pack_w8(w):
        wp_ = (np.asarray(w)[perm] * (WS * rowscale[:, None])).T  # [K, 1024]
        K = wp_.shape[0]
        arr = wp_.reshape(K // 128, 128, 1024).transpose(1, 0, 2)  # [p, j, m]
        return arr.reshape(128, -1).astype(E4NP)

    def pack_wx(wih, btot):
        rows = np.concatenate([np.asarray(wih)[perm].T,
                               np.asarray(btot)[perm][None, :]], axis=0)  # [4, 1024]
        rows = rows * (WS * HS * rowscale[None, :])
        return rows.astype(np.float16)

    shared = {}
    shared["whh_e0"] = pack_w8(inputs["enc_Whh0"])
    shared["wih_e1"] = pack_w8(inputs["enc_Wih1"])
    shared["whh_e1"] = pack_w8(inputs["enc_Whh1"])
    shared["whh_d0"] = pack_w8(inputs["dec_Whh0"])
    shared["wih_d1"] = pack_w8(inputs["dec_Wih1"])
    shared["whh_d1"] = pack_w8(inputs["dec_Whh1"])
    shared["wx_e0"] = pack_wx(inputs["enc_Wih0"], inputs["enc_bih0"] + inputs["enc_bhh0"])
    shared["wx_d0"] = pack_wx(inputs["dec_Wih0"], inputs["dec_bih0"] + inputs["dec_bhh0"])

    bias1 = np.zeros((128, 16), np.float32)
    for j, pre in enumerate(("enc_", "dec_")):
        b = (np.asarray(inputs[pre + "bih1"]) + np.asarray(inputs[pre + "bhh1"]))[perm]
        b = b * rowscale
        bias1[:, j * 8:(j + 1) * 8] = b.reshape(8, 128).T
    shared["bias1"] = bias1

    fcw = np.zeros((256, 4), np.float32)
    fcw[:, :3] = np.asarray(inputs["fc_W"]).T * FS
    shared["fcw"] = fcw.reshape(2, 128, 4).transpose(1, 0, 2).reshape(128, 8).astype(E4NP)
    shared["fcbbm"] = np.tile(np.asarray(inputs["fc_b"]).astype(np.float32), (128, NB)).copy()
    shared["magic"] = np.full((128, NB), RSQRT_MAGIC, np.uint32)
    shared["ident"] = np.eye(128, dtype=np.float32)

    enc = np.asarray(inputs["encoder_position_inputs"], np.float32)
    dec = np.asarray(inputs["decoder_position_inputs"], np.float32)
    in_maps = []
    for c in range(N_CORES):
        sl = slice(c * B, (c + 1) * B)
        m = dict(shared)
        ex = enc[sl].transpose(2, 1, 0).reshape(3, T_ENC * B)
        m["enc_x"] = np.concatenate([ex, np.ones((1, T_ENC * B), np.float32)],
                                    axis=0).astype(np.float16)
        p0 = dec[sl, 0, :]
        m["pos0"] = np.concatenate([p0.T, np.ones((1, B), np.float32)],
                                   axis=0).astype(np.float16)
        m["pos0bm"] = p0.reshape(NB, 128, 3).transpose(1, 0, 2).reshape(128, NB * 3).astype(np.float32).copy()
        in_maps.append(m)
    return in_maps


def unpack_outputs(results):
    outs = []
    for c in range(N_CORES):
        ys = results[c]["ys"].reshape(128, T_DEC, NB, 3)
        outs.append(ys.transpose(2, 0, 1, 3).reshape(B, T_DEC, 3))
    return np.concatenate(outs, axis=0)


class SpmdRunner:
    """Compile a finalized Bass module once; run it many times."""

    def __init__(self, nc, n_cores):
        import jax
        from jax.sharding import Mesh, PartitionSpec
        from jax.experimental.shard_map import shard_map
        from concourse.bass2jax import _bass_exec_p, install_neuronx_cc_hook, partition_id_tensor
        self.jax = jax
        self.PartitionSpec = PartitionSpec
        install_neuronx_cc_hook()
        self.nc = nc
        self.n_cores = n_cores
        partition_name = nc.partition_id_tensor.name if nc.partition_id_tensor else None
        in_names, out_names, out_avals = [], [], []
        for alloc in nc.m.functions[0].allocations:
            if not isinstance(alloc, mybir.MemoryLocationSet):
                continue
            name = alloc.memorylocations[0].name
            if alloc.kind == "ExternalInput":
                if name != partition_name:
                    in_names.append(name)
            elif alloc.kind == "ExternalOutput":
                out_names.append(name)
                out_avals.append(jax.core.ShapedArray(tuple(alloc.tensor_shape), mybir.dt.np(alloc.dtype)))
        self.in_names, self.out_names, self.out_avals = in_names, out_names, out_avals
        n_params = len(in_names)
        n_outs = len(out_avals)
        all_in_names = list(in_names) + list(out_names)
        if partition_name is not None:
            all_in_names.append(partition_name)

        def _body(*args):
            operands = list(args)
            if partition_name is not None:
                operands.append(partition_id_tensor())
            outs = _bass_exec_p.bind(
                *operands,
                out_avals=tuple(out_avals),
                in_names=tuple(all_in_names),
                out_names=tuple(out_names),
                lowering_input_output_aliases=(),
                sim_require_finite=True,
                sim_require_nnan=True,
                nc=nc,
            )
            return tuple(outs)

        devices = jax.devices()[:n_cores]
        self.mesh = Mesh(np.asarray(devices), ("core",))
        in_specs = (PartitionSpec("core"),) * (n_params + n_outs)
        out_specs = (PartitionSpec("core"),) * n_outs
        donate = tuple(range(n_params, n_params + n_outs))
        self.sharded = jax.jit(
            shard_map(_body, mesh=self.mesh, in_specs=in_specs, out_specs=out_specs, check_rep=False),
            donate_argnums=donate, keep_unused=True,
        )
        self.n_params, self.n_outs = n_params, n_outs

    def __call__(self, in_maps, n_timed=0):
        import time
        jax = self.jax
        from jax.sharding import NamedSharding
        per_core = [[np.asarray(m[name]) for name in self.in_names] for m in in_maps]
        concat_in = [np.concatenate([per_core[c][i] for c in range(self.n_cores)], axis=0)
                     for i in range(self.n_params)]
        sh = NamedSharding(self.mesh, self.PartitionSpec("core"))
        concat_in = [jax.device_put(a, sh) for a in concat_in]

        def zeros():
            return [jax.device_put(np.zeros((self.n_cores * a.shape[0], *a.shape[1:]), a.dtype), sh)
                    for a in self.out_avals]

        out_arrs = jax.block_until_ready(self.sharded(*concat_in, *zeros()))
        times = []
        for _ in range(n_timed):
            z = zeros()
            jax.block_until_ready(z)
            t0 = time.perf_counter()
            out_arrs = jax.block_until_ready(self.sharded(*concat_in, *z))
            times.append(time.perf_counter() - t0)
        results = [
            {name: np.asarray(out_arrs[i]).reshape(self.n_cores, *self.out_avals[i].shape)[c]
             for i, name in enumerate(self.out_names)}
            for c in range(self.n_cores)
        ]
        return results, times


_RUNNER_CACHE = {}


def get_runner(repeats=1):
    if repeats not in _RUNNER_CACHE:
        _RUNNER_CACHE[repeats] = SpmdRunner(build_kernel(repeats=repeats), N_CORES)
    return _RUNNER_CACHE[repeats]


def kernel(**inputs) -> np.ndarray:
    run = get_runner(repeats=1)
    in_maps = pack_inputs(inputs)
    results, _ = run(in_maps)
    return unpack_outputs(results).astype(np.float32)


# revision 5
# speedup vs baseline: 1.0301x; 1.0301x over previous
"""TRN2 Bass kernel for nn_PosOnlyModel: 2-layer LSTM encoder (15 steps) +
autoregressive 2-layer LSTM decoder (25 steps) with Linear head and
unit-sphere residual position updates.

Pure data parallel across 8 NeuronCores (batch 8192 -> 1024/core, weights
replicated). Per-core structure:
- Recurrent matmuls (Whh0, Wih1, Whh1) run in fp8e4 DoubleRow mode: K=256 in
  ONE PE pass (half the streaming cycles and half the matmul dispatch of an
  fp16 2-pass kernel). Weights are packed x16 and h stored as 64*h in fp8;
  the gate activation applies scale=1/1024 for free.
- Layer-0 bias folds into the K=4 fp16 x-pass via a ones row in the x tile.
- g-gate rows carry an extra x2 so every gate uses Sigmoid; tanh(g) is
  recovered as 2*sigmoid(2g)-1 by one DVE tensor_scalar (4x mode). The
  g sigmoid is kept fp32 to avoid cancellation noise near 0.5.
- h = (64*sigmoid(o)) * tanh(c): the 64*sigmoid(o) tensor_scalar runs off
  the critical path, so the step tail is ACT-tanh + one TT into fp8.
- Per-gate PSUM tiles [128,1024] (2 banks, bufs=4) pipeline PE->ACT;
  element-wise cell updates run in 512-wide chunks so consecutive steps
  overlap across engines. The whole c-update (m1, tg, m2, add) stays on DVE
  in fp16 - keeping it on one engine removes cross-engine semaphore hops
  from the recurrence-critical path (moving m1 to Pool measured 6% slower).
- Decoder position chain: batch-major [128, 8, 3] DVE ops with bit-trick
  Newton rsqrt (ACT Sqrt would force a ~1.3us activation-table swap per
  step), PE transposes back to [3, B], and the PSUM->SBUF evacuation split
  across ACT and DVE.
"""
import sys
import numpy as np
import ml_dtypes

sys.path.insert(0, '/opt/trn_rl_repo')

import concourse.bass as bass
import concourse.bacc as bacc
import concourse.mybir as mybir
from concourse.tile import TileContext

dt = mybir.dt
F32 = dt.float32
F16 = dt.float16
E4 = dt.float8e4
U32 = dt.uint32
AF = mybir.ActivationFunctionType
ALU = mybir.AluOpType
AX = mybir.AxisListType
DR = mybir.MatmulPerfMode.DoubleRow

H = 256
T_ENC = 15
T_DEC = 25
N_CORES = 8
B_FULL = 8192
B = B_FULL // N_CORES
NB = B // 128
NK = 2
NC = 2          # batch chunks for element-wise pipelining
CW = B // NC    # chunk width
WS = 16.0       # weight scale into fp8 range
HS = 64.0       # h scale into fp8 range (h stored as 64*h)
FS = 16.0       # fc weight scale
RSQRT_MAGIC = 0x5f3759df
E4NP = ml_dtypes.float8_e4m3


def build_kernel(repeats=1):
    nc = bacc.Bacc("TRN2", target_bir_lowering=False)

    enc_x = nc.declare_dram_parameter("enc_x", [4, T_ENC * B], F16, isOutput=False)
    pos0 = nc.declare_dram_parameter("pos0", [4, B], F16, isOutput=False)
    pos0bm = nc.declare_dram_parameter("pos0bm", [128, NB * 3], F32, isOutput=False)
    w8_d = {}
    for name in ("whh_e0", "wih_e1", "whh_e1", "whh_d0", "wih_d1", "whh_d1"):
        w8_d[name] = nc.declare_dram_parameter(name, [128, NK * 8 * 128], E4, isOutput=False)
    wx_d = {c: nc.declare_dram_parameter(f"wx_{c}", [4, 8 * 128], F16, isOutput=False)
            for c in ("e0", "d0")}
    bias1_d = nc.declare_dram_parameter("bias1", [128, 16], F32, isOutput=False)
    fcw_d = nc.declare_dram_parameter("fcw", [128, NK * 4], E4, isOutput=False)
    fcbbm_d = nc.declare_dram_parameter("fcbbm", [128, NB * 3], F32, isOutput=False)
    magic_d = nc.declare_dram_parameter("magic", [128, NB], U32, isOutput=False)
    ident_d = nc.declare_dram_parameter("ident", [128, 128], F32, isOutput=False)
    ys_d = nc.declare_dram_parameter("ys", [128, T_DEC * NB * 3], F32, isOutput=True)

    with TileContext(nc) as tc:
        with tc.tile_pool(name="wpool", bufs=1) as wp, \
             tc.tile_pool(name="state", bufs=1) as sp, \
             tc.tile_pool(name="acts", bufs=12) as ap, \
             tc.tile_pool(name="tanhs", bufs=4) as t3p, \
             tc.tile_pool(name="tmps", bufs=8) as tp, \
             tc.tile_pool(name="xp", bufs=3) as xp, \
             tc.tile_pool(name="dec", bufs=3) as dp, \
             tc.tile_pool(name="psum", bufs=4, space="PSUM") as pp:

            w8 = {}
            for name in w8_d:
                w8[name] = wp.tile([128, NK, 8 * 128], E4, name=name)
                nc.sync.dma_start(out=w8[name][:],
                                  in_=w8_d[name][:].rearrange("p (k m) -> p k m", k=NK))
            wx = {}
            for c in ("e0", "d0"):
                wx[c] = wp.tile([4, 8 * 128], F16, name=f"wx_{c}")
                nc.gpsimd.dma_start(out=wx[c][:], in_=wx_d[c][:])
            bias1_t = wp.tile([128, 16], F32, name="bias1")
            nc.gpsimd.dma_start(out=bias1_t[:], in_=bias1_d[:])
            fcw_t = wp.tile([128, NK, 4], E4, name="fcw")
            nc.gpsimd.dma_start(out=fcw_t[:], in_=fcw_d[:].rearrange("p (k m) -> p k m", k=NK))
            fcbbm_t = wp.tile([128, NB * 3], F32, name="fcbbm")
            nc.gpsimd.dma_start(out=fcbbm_t[:], in_=fcbbm_d[:])
            magic_t = wp.tile([128, NB], U32, name="magic")
            nc.gpsimd.dma_start(out=magic_t[:], in_=magic_d[:])
            ident_t = wp.tile([128, 128], F32, name="ident")
            nc.gpsimd.dma_start(out=ident_t[:], in_=ident_d[:])
            pos0bm_t = wp.tile([128, NB * 3], F32, name="pos0bm")
            nc.gpsimd.dma_start(out=pos0bm_t[:], in_=pos0bm[:])
            posx = [wp.tile([4, B], F16, name=f"posx{i}") for i in range(2)]
            for t in posx:
                nc.gpsimd.dma_start(out=t[3:4, :], in_=pos0[3:4, :])

            h_t = [[sp.tile([128, NK, B], E4, name=f"h{l}_{par}") for par in range(2)]
                   for l in range(2)]
            c_t = [sp.tile([128, NK, B], F16, name=f"c{l}") for l in range(2)]

            CELLS = {0: ("e0", "e1"), 1: ("d0", "d1")}

            def emit_cell(cell, layer, s, x_ap, dr_rd, h_wr, c_rw, bias_base):
                """dr_rd: list of (w8_tile, h_rhs_tile) DoubleRow passes.
                Layer 0 adds the fp16 x-pass (bias folded via ones row)."""
                first = s == 0
                ga = {}  # gate -> act tile AP, key (hb, mg)
                so64 = {}
                for hb in range(2):
                    # f first (Pool m1 overlaps), g second (tg->m2->add chain
                    # starts early), o last (h-mul is its only consumer)
                    for mg in (1, 2, 0, 3):
                        m = hb * 4 + mg
                        msl = slice(m * 128, (m + 1) * 128)
                        ps = pp.tile([128, B], F32, name="ps", tag="ps")
                        n_pass = len(dr_rd) + (1 if layer == 0 else 0)
                        pj = 0
                        for (w, rhs) in dr_rd:
                            for ci in range(NC):
                                cs = slice(ci * CW, (ci + 1) * CW)
                                nc.tensor.matmul(ps[:, cs], w[:, :, msl], rhs[:, :, cs],
                                                 start=(pj == 0), stop=(pj == n_pass - 1),
                                                 perf_mode=DR)
                            pj += 1
                        if layer == 0:
                            for ci in range(NC):
                                cs = slice(ci * CW, (ci + 1) * CW)
                                nc.tensor.matmul(ps[:, cs], wx[cell][0:4, msl],
                                                 x_ap[0:4, cs],
                                                 start=(pj == 0), stop=True)
                        # g gate gets Tanh directly (same ACT table set as
                        # Sigmoid, so no table swap) - the gate activations
                        # feed the DVE cell update with no fix-up ops
                        a = ap.tile([128, B], F16, name="act", tag="act")
                        func = AF.Tanh if mg == 2 else AF.Sigmoid
                        if layer == 0:
                            nc.scalar.activation(a[:], ps[:], func,
                                                 scale=1.0 / (WS * HS))
                        else:
                            col = bias_base + m
                            nc.scalar.activation(a[:], ps[:], func,
                                                 bias=bias1_t[:, col:col + 1],
                                                 scale=1.0 / (WS * HS))
                        ga[(hb, mg)] = a
                    # chunked cell update for this half
                    for ci in range(NC):
                        cs = slice(ci * CW, (ci + 1) * CW)
                        cd = c_rw[:, hb, cs]
                        if first:
                            nc.vector.tensor_mul(cd, ga[(hb, 0)][:, cs],
                                                 ga[(hb, 2)][:, cs])
                        else:
                            # whole c-update on DVE: f activates first so m1
                            # is ready early, and one engine means no cross-
                            # engine semaphore hops on the recurrence path
                            m1 = tp.tile([128, CW], F16, name="m1", tag="m1")
                            nc.vector.tensor_mul(m1[:], ga[(hb, 1)][:, cs], cd)
                            m2 = tp.tile([128, CW], F16, name="m2", tag="m2")
                            nc.vector.tensor_mul(m2[:], ga[(hb, 0)][:, cs],
                                                 ga[(hb, 2)][:, cs])
                            nc.vector.tensor_add(cd, m1[:], m2[:])
                for hb in range(2):
                    # HS*sig(o) off the critical path (exact: HS is 2^6)
                    for ci in range(NC):
                        cs = slice(ci * CW, (ci + 1) * CW)
                        so = tp.tile([128, CW], F16, name="so64", tag="so64")
                        nc.vector.tensor_scalar(out=so[:], in0=ga[(hb, 3)][:, cs],
                                                scalar1=HS, scalar2=None, op0=ALU.mult)
                        so64[(hb, ci)] = so
                # h = (HS*sig(o)) * tanh(c) -> fp8 stores 64h; tanh feeds the
                # h-mul directly so the step tail is just ACT-tanh + one TT
                for ci in range(NC):
                    cs = slice(ci * CW, (ci + 1) * CW)
                    th = t3p.tile([128, NK, CW], F16, name="th", tag="th")
                    nc.scalar.activation(th[:], c_rw[:, :, cs], AF.Tanh)
                    for hb in range(2):
                        nc.vector.tensor_mul(h_wr[:, hb, cs], so64[(hb, ci)][:],
                                             th[:, hb, :])

            for rep in range(repeats):
                pos_bm = None
                for s in range(T_ENC + T_DEC):
                    is_dec = s >= T_ENC
                    d = s - T_ENC
                    p, q = s % 2, 1 - s % 2
                    c0n, c1n = CELLS[1 if is_dec else 0]
                    first = s == 0
                    if is_dec:
                        if d == 0:
                            nc.sync.dma_start(out=posx[0][:], in_=pos0[:])
                            pos_bm = dp.tile([128, NB * 3], F32, name="pbm", tag="pbm")
                            nc.vector.tensor_add(pos_bm[:], pos0bm_t[:], fcbbm_t[:])
                        x_ap = posx[d % 2][:]
                    else:
                        xt = xp.tile([4, B], F16, name="xst", tag="xst")
                        nc.sync.dma_start(out=xt[:], in_=enc_x[:, s * B:(s + 1) * B])
                        x_ap = xt[:]

                    dr0 = [] if first else [(w8["whh_" + c0n], h_t[0][q])]
                    emit_cell(c0n, 0, s, x_ap, dr0, h_t[0][p], c_t[0], 0)
                    # whh pass first: h1(s-1) is available long before h0(s),
                    # so the PE can fill the psum group early
                    dr1 = []
                    if not first:
                        dr1.append((w8["whh_" + c1n], h_t[1][q]))
                    dr1.append((w8["wih_" + c1n], h_t[0][p]))
                    emit_cell(c1n, 1, s, None, dr1, h_t[1][p], c_t[1],
                              0 if not is_dec else 8)

                    if is_dec:
                        psd = pp.tile([128, NB, 4], F32, name="psd", tag="ps")
                        for m in range(NB):
                            for k in range(NK):
                                nc.tensor.matmul(psd[:, m, :],
                                                 h_t[1][p][:, k, m * 128:(m + 1) * 128],
                                                 fcw_t[:, k, :],
                                                 start=(k == 0), stop=(k == NK - 1))
                        nd = dp.tile([128, NB * 3], F32, name="nd", tag="nd")
                        nc.vector.tensor_scalar(
                            out=nd[:].rearrange("p (m c) -> p m c", c=3),
                            in0=psd[:, :, 0:3], scalar1=1.0 / (HS * FS), scalar2=None,
                            op0=ALU.mult)
                        npos = dp.tile([128, NB * 3], F32, name="npos", tag="npos")
                        nc.vector.tensor_add(npos[:], nd[:], pos_bm[:])
                        sq = dp.tile([128, NB * 3], F32, name="sq", tag="sq")
                        nc.vector.tensor_mul(sq[:], npos[:], npos[:])
                        ss = dp.tile([128, NB], F32, name="ss", tag="ss")
                        nc.vector.tensor_reduce(ss[:], sq[:].rearrange("p (m c) -> p m c", c=3),
                                                axis=AX.X, op=ALU.add)
                        # rsqrt via bit trick + 2 Newton iters. (ACT Sqrt is
                        # accurate but lives in another activation-table set:
                        # using it costs 2 x ~1.3us table loads per step.)
                        u1 = dp.tile([128, NB], U32, name="u1", tag="u1")
                        nc.vector.tensor_scalar(out=u1[:], in0=ss[:].bitcast(U32),
                                                scalar1=1, scalar2=None,
                                                op0=ALU.logical_shift_right)
                        y = dp.tile([128, NB], F32, name="y", tag="y")
                        nc.vector.tensor_tensor(out=y[:].bitcast(U32), in0=magic_t[:],
                                                in1=u1[:], op=ALU.subtract)
                        for _ in range(2):
                            t = dp.tile([128, NB], F32, name="nrt", tag="nrt")
                            nc.vector.tensor_mul(t[:], y[:], y[:])
                            nc.vector.tensor_mul(t[:], t[:], ss[:])
                            nc.vector.tensor_scalar(out=t[:], in0=t[:], scalar1=-0.5,
                                                    scalar2=1.5, op0=ALU.mult, op1=ALU.add)
                            nc.vector.tensor_mul(y[:], y[:], t[:])
                        posn = dp.tile([128, NB * 3], F32, name="posn", tag="posn")
                        yb = y[:].unsqueeze(2).broadcast_to([128, NB, 3])
                        nc.vector.tensor_tensor(out=posn[:].rearrange("p (m c) -> p m c", c=3),
                                                in0=npos[:].rearrange("p (m c) -> p m c", c=3),
                                                in1=yb, op=ALU.mult)
                        nc.sync.dma_start(out=ys_d[:, d * NB * 3:(d + 1) * NB * 3], in_=posn[:])
                        if d < T_DEC - 1:
                            pbm2 = dp.tile([128, NB * 3], F32, name="pbm", tag="pbm")
                            nc.vector.tensor_add(pbm2[:], posn[:], fcbbm_t[:])
                            pos_bm = pbm2
                            ps_tr = pp.tile([3, B], F32, name="ps_tr", tag="ps")
                            for m in range(NB):
                                nc.tensor.transpose(ps_tr[:, m * 128:(m + 1) * 128],
                                                    posn[:, m * 3:(m + 1) * 3], ident_t[:])
                            # split the PSUM->SBUF evacuation across ACT+DVE
                            nxt = posx[(d + 1) % 2]
                            nc.scalar.copy(out=nxt[0:3, 0:CW], in_=ps_tr[:, 0:CW])
                            nc.vector.tensor_copy(out=nxt[0:3, CW:B], in_=ps_tr[:, CW:B])

    nc.finalize()
    return nc


def pack_inputs(inputs):
    perm = np.concatenate([np.arange(g * 256 + hb * 128, g * 256 + hb * 128 + 128)
                           for hb in range(2) for g in range(4)])
    rowscale = np.ones(4 * H, np.float32)

    def pack_w8(w):
        wp_ = (np.asarray(w)[perm] * (WS * rowscale[:, None])).T  # [K, 1024]
        K = wp_.shape[0]
        arr = wp_.reshape(K // 128, 128, 1024).transpose(1, 0, 2)  # [p, j, m]
        return arr.reshape(128, -1).astype(E4NP)

    def pack_wx(wih, btot):
        rows = np.concatenate([np.asarray(wih)[perm].T,
                               np.asarray(btot)[perm][None, :]], axis=0)  # [4, 1024]
        rows = rows * (WS * HS * rowscale[None, :])
        return rows.astype(np.float16)

    shared = {}
    shared["whh_e0"] = pack_w8(inputs["enc_Whh0"])
    shared["wih_e1"] = pack_w8(inputs["enc_Wih1"])
    shared["whh_e1"] = pack_w8(inputs["enc_Whh1"])
    shared["whh_d0"] = pack_w8(inputs["dec_Whh0"])
    shared["wih_d1"] = pack_w8(inputs["dec_Wih1"])
    shared["whh_d1"] = pack_w8(inputs["dec_Whh1"])
    shared["wx_e0"] = pack_wx(inputs["enc_Wih0"], inputs["enc_bih0"] + inputs["enc_bhh0"])
    shared["wx_d0"] = pack_wx(inputs["dec_Wih0"], inputs["dec_bih0"] + inputs["dec_bhh0"])

    bias1 = np.zeros((128, 16), np.float32)
    for j, pre in enumerate(("enc_", "dec_")):
        b = (np.asarray(inputs[pre + "bih1"]) + np.asarray(inputs[pre + "bhh1"]))[perm]
        b = b * rowscale
        bias1[:, j * 8:(j + 1) * 8] = b.reshape(8, 128).T
    shared["bias1"] = bias1

    fcw = np.zeros((256, 4), np.float32)
    fcw[:, :3] = np.asarray(inputs["fc_W"]).T * FS
    shared["fcw"] = fcw.reshape(2, 128, 4).transpose(1, 0, 2).reshape(128, 8).astype(E4NP)
    shared["fcbbm"] = np.tile(np.asarray(inputs["fc_b"]).astype(np.float32), (128, NB)).copy()
    shared["magic"] = np.full((128, NB), RSQRT_MAGIC, np.uint32)
    shared["ident"] = np.eye(128, dtype=np.float32)

    enc = np.asarray(inputs["encoder_position_inputs"], np.float32)
    dec = np.asarray(inputs["decoder_position_inputs"], np.float32)
    in_maps = []
    for c in range(N_CORES):
        sl = slice(c * B, (c + 1) * B)
        m = dict(shared)
        ex = enc[sl].transpose(2, 1, 0).reshape(3, T_ENC * B)
        m["enc_x"] = np.concatenate([ex, np.ones((1, T_ENC * B), np.float32)],
                                    axis=0).astype(np.float16)
        p0 = dec[sl, 0, :]
        m["pos0"] = np.concatenate([p0.T, np.ones((1, B), np.float32)],
                                   axis=0).astype(np.float16)
        m["pos0bm"] = p0.reshape(NB, 128, 3).transpose(1, 0, 2).reshape(128, NB * 3).astype(np.float32).copy()
        in_maps.append(m)
    return in_maps


def unpack_outputs(results):
    outs = []
    for c in range(N_CORES):
        ys = results[c]["ys"].reshape(128, T_DEC, NB, 3)
        outs.append(ys.transpose(2, 0, 1, 3).reshape(B, T_DEC, 3))
    return np.concatenate(outs, axis=0)


class SpmdRunner:
    """Compile a finalized Bass module once; run it many times."""

    def __init__(self, nc, n_cores):
        import jax
        from jax.sharding import Mesh, PartitionSpec
        from jax.experimental.shard_map import shard_map
        from concourse.bass2jax import _bass_exec_p, install_neuronx_cc_hook, partition_id_tensor
        self.jax = jax
        self.PartitionSpec = PartitionSpec
        install_neuronx_cc_hook()
        self.nc = nc
        self.n_cores = n_cores
        partition_name = nc.partition_id_tensor.name if nc.partition_id_tensor else None
        in_names, out_names, out_avals = [], [], []
        for alloc in nc.m.functions[0].allocations:
            if not isinstance(alloc, mybir.MemoryLocationSet):
                continue
            name = alloc.memorylocations[0].name
            if alloc.kind == "ExternalInput":
                if name != partition_name:
                    in_names.append(name)
            elif alloc.kind == "ExternalOutput":
                out_names.append(name)
                out_avals.append(jax.core.ShapedArray(tuple(alloc.tensor_shape), mybir.dt.np(alloc.dtype)))
        self.in_names, self.out_names, self.out_avals = in_names, out_names, out_avals
        n_params = len(in_names)
        n_outs = len(out_avals)
        all_in_names = list(in_names) + list(out_names)
        if partition_name is not None:
            all_in_names.append(partition_name)

        def _body(*args):
            operands = list(args)
            if partition_name is not None:
                operands.append(partition_id_tensor())
            outs = _bass_exec_p.bind(
                *operands,
                out_avals=tuple(out_avals),
                in_names=tuple(all_in_names),
                out_names=tuple(out_names),
                lowering_input_output_aliases=(),
                sim_require_finite=True,
                sim_require_nnan=True,
                nc=nc,
            )
            return tuple(outs)

        devices = jax.devices()[:n_cores]
        self.mesh = Mesh(np.asarray(devices), ("core",))
        in_specs = (PartitionSpec("core"),) * (n_params + n_outs)
        out_specs = (PartitionSpec("core"),) * n_outs
        donate = tuple(range(n_params, n_params + n_outs))
        self.sharded = jax.jit(
            shard_map(_body, mesh=self.mesh, in_specs=in_specs, out_specs=out_specs, check_rep=False),
            donate_argnums=donate, keep_unused=True,
        )
        self.n_params, self.n_outs = n_params, n_outs

    def __call__(self, in_maps, n_timed=0):
        import time
        jax = self.jax
        from jax.sharding import NamedSharding
        per_core = [[np.asarray(m[name]) for name in self.in_names] for m in in_maps]
        concat_in = [np.concatenate([per_core[c][i] for c in range(self.n_cores)], axis=0)
                     for i in range(self.n_params)]
        sh = NamedSharding(self.mesh, self.PartitionSpec("core"))
        concat_in = [jax.device_put(a, sh) for a in concat_in]

        def zeros():
            return [jax.device_put(np.zeros((self.n_cores * a.shape[0], *a.shape[1:]), a.dtype), sh)
                    for a in self.out_avals]

        out_arrs = jax.block_until_ready(self.sharded(*concat_in, *zeros()))
        times = []
        for _ in range(n_timed):
            z = zeros()
            jax.block_until_ready(z)
            t0 = time.perf_counter()
            out_arrs = jax.block_until_ready(self.sharded(*concat_in, *z))
            times.append(time.perf_counter() - t0)
        results = [
            {name: np.asarray(out_arrs[i]).reshape(self.n_cores, *self.out_avals[i].shape)[c]
             for i, name in enumerate(self.out_names)}
            for c in range(self.n_cores)
        ]
        return results, times


_RUNNER_CACHE = {}


def get_runner(repeats=1):
    if repeats not in _RUNNER_CACHE:
        _RUNNER_CACHE[repeats] = SpmdRunner(build_kernel(repeats=repeats), N_CORES)
    return _RUNNER_CACHE[repeats]


def kernel(**inputs) -> np.ndarray:
    run = get_runner(repeats=1)
    in_maps = pack_inputs(inputs)
    results, _ = run(in_maps)
    return unpack_outputs(results).astype(np.float32)
